# revision 18
# baseline (speedup 1.0000x reference)
"""Multi-head attention (B=8, T=2048, C=256, H=4) on 8 NeuronCores.

Data-parallel over batch: core b computes batch element b end-to-end.

Dataflow (transposed so contraction dims land on SBUF partitions):
  xT [C,T] from x via PE transpose (tt0-5) + DMA-xbar transpose (tt6-15)
  qkT [2C,T] = w_qk @ xT + b_qk ; v [T,H,65] = x @ w_v.T + b_v (+ones col)
  scores per (q-tile 512, head-pair): K=64 matmuls row-packed h0/h1,
  exp on ScalarE straight from PSUM (GROUP=3 steps per ACTIVATE, FD=1536,
  amortizing the ~350-cycle instruction overhead), PV accumulates
  out2T[65,512] over 16 k-chunks -- split into K=64 half-matmuls in an
  (A||D),(B||C) row-group/bank pattern so LDWEIGHTS hides; sumexp rides
  the ones column; deferred normalization via one batched reciprocal +
  K=1 broadcast matmuls; proj with yT stationary -> natural [T,C] out.

Scheduling: ScalarE(exp) is the roofline (~132us of ACTIVATE). All other
work is pipelined under it:
  - qk/v projections + remaining transposes spread across (qt=0,*) chunks
  - norm + proj of q-tile qt run as fillers during (qt+1, 0)
  - PV matmuls of exp-group g are emitted after the scores of group g+1
    (carried across (qt,hp) boundaries) so the PE never waits on ACT
  - qt=3/hp=0 normalization runs inside the (3,1) stream to shrink the
    tail; only hp=1 norm + 4 proj chunks remain after the last exp.
"""

import numpy as np

import concourse.bass as bass
import concourse.tile as tile
from concourse import bacc, mybir
from concourse.bass_utils import run_bass_kernel_spmd
from concourse.masks import make_identity

B, T, C = 8, 2048, 256
H, HD = 4, 64
N_CORES = 8
F32 = mybir.dt.float32
F32R = mybir.dt.float32r
BF16 = mybir.dt.bfloat16

QT = 512                # q-tile (columns of scoresT per inner iteration)
NQT = T // QT           # 4
KC = T // 128           # 16 k-chunks of 128
GROUP = 3               # (chunk, head) steps per ACTIVATE
USE_DMA_TRANS = False   # move transposes tt6-15 to the DMA xbar
SPLIT_PV = False         # K=64 half-matmul PV disabled (bisect)


def build_nc():
    nc = bacc.Bacc("TRN2", target_bir_lowering=False, debug=False,
                   num_devices=N_CORES)

    x_ap = nc.dram_tensor("x", [T, C], F32, kind="ExternalInput").ap()
    wqk_ap = nc.dram_tensor("w_qkT", [C, 2 * C], F32, kind="ExternalInput").ap()
    wv_ap = nc.dram_tensor("w_vT", [C, C], F32, kind="ExternalInput").ap()
    wp_ap = nc.dram_tensor("w_pT", [C, C], F32, kind="ExternalInput").ap()
    bqk_ap = nc.dram_tensor("b_qk", [4, 128], F32, kind="ExternalInput").ap()
    bv_ap = nc.dram_tensor("b_v", [C], F32, kind="ExternalInput").ap()
    bp_ap = nc.dram_tensor("b_p", [C], F32, kind="ExternalInput").ap()
    out_ap = nc.dram_tensor("out", [T, C], F32, kind="ExternalOutput").ap()

    with tile.TileContext(nc) as tc:
        with (
            tc.tile_pool(name="consts", bufs=1) as consts,
            tc.tile_pool(name="xstage", bufs=4) as xstage,
            tc.tile_pool(name="xt", bufs=1) as xtp,
            tc.tile_pool(name="qkt", bufs=1) as qktp,
            tc.tile_pool(name="vsb", bufs=1) as vsbp,
            tc.tile_pool(name="expp", bufs=4) as expp,
            tc.tile_pool(name="yt", bufs=1) as ytp,
            tc.tile_pool(name="ostage", bufs=4) as ostage,
            tc.tile_pool(name="small", bufs=6) as small,
            tc.tile_pool(name="scps", bufs=2, space="PSUM") as scps,
            tc.tile_pool(name="o2ps", bufs=1, space="PSUM") as o2ps,
        ):
            # ---- constants / weights -------------------------------------
            ident = consts.tile([128, 128], BF16, tag="ident")
            make_identity(nc, ident[:])

            ones_f = consts.tile([97, 64], F32, tag="ones_f")
            nc.vector.memset(ones_f[:], 1.0)
            ones_r = consts.tile([97, 64], F32R, tag="ones_r")
            nc.vector.tensor_copy(ones_r[:], ones_f[:])

            onescol = consts.tile([128, H], F32, tag="onescol")
            nc.vector.memset(onescol[:], 1.0)

            # x (casting loads) alone on the gpsimd ring: the first chunk in
            # per-tt slices so transposes start at ~1.5us. Weights load as
            # plain fp32 on the sync ring and are cast to bf16 on the DVE,
            # so the two rings run in parallel at startup.
            x_re = x_ap.rearrange("(b a p) c -> b p a c", b=4, p=128)
            xsbig = [xstage.tile([128, 4, C], BF16, tag="xs", name=f"xs{b}")
                     for b in range(4)]
            for a in range(4):
                nc.gpsimd.dma_start(xsbig[0][:, a:a + 1, :], x_re[0][:, a:a + 1, :])
            for b in range(1, 4):
                nc.gpsimd.dma_start(xsbig[b][:], x_re[b])

            w_qk = [consts.tile([128, 2 * C], BF16, tag=f"wqk{c}", name=f"wqk{c}") for c in range(2)]
            w_v = [consts.tile([128, C], BF16, tag=f"wv{c}", name=f"wv{c}") for c in range(2)]
            w_p = [consts.tile([128, C], BF16, tag=f"wp{c}", name=f"wp{c}") for c in range(2)]
            wqk_f = [consts.tile([128, 2 * C], F32, tag=f"wqkf{c}", name=f"wqkf{c}") for c in range(2)]
            wv_f = [consts.tile([128, C], F32, tag=f"wvf{c}", name=f"wvf{c}") for c in range(2)]
            wp_f = [consts.tile([128, C], F32, tag=f"wpf{c}", name=f"wpf{c}") for c in range(2)]
            bqk_nat = consts.tile([4, 128], F32, tag="bqkn")
            nc.sync.dma_start(bqk_nat[:], bqk_ap)
            for c in range(2):
                nc.sync.dma_start(wqk_f[c][:], wqk_ap[128 * c:128 * (c + 1), :])
                nc.vector.tensor_copy(w_qk[c][:], wqk_f[c][:])
            for c in range(2):
                nc.sync.dma_start(wv_f[c][:], wv_ap[128 * c:128 * (c + 1), :])
                nc.vector.tensor_copy(w_v[c][:], wv_f[c][:])
            for c in range(2):
                nc.sync.dma_start(wp_f[c][:], wp_ap[128 * c:128 * (c + 1), :])
                nc.vector.tensor_copy(w_p[c][:], wp_f[c][:])
            b_p = consts.tile([128, C], F32, tag="bp")
            bp_bc = bass.AP(tensor=bp_ap.tensor, offset=bp_ap.offset,
                            ap=[[0, 128]] + list(bp_ap.ap))
            nc.sync.dma_start(b_p[:], bp_bc)
            b_v = consts.tile([128, C], F32, tag="bv")
            bv_bc = bass.AP(tensor=bv_ap.tensor, offset=bv_ap.offset,
                            ap=[[0, 128]] + list(bv_ap.ap))
            nc.sync.dma_start(b_v[:], bv_bc)

            # ---- persistent SBUF tensors ---------------------------------
            xt = [xtp.tile([128, T], BF16, tag=f"xt{c}", name=f"xt{c}") for c in range(2)]
            qkt = [qktp.tile([128, T], BF16, tag=f"qkt{m}", name=f"qkt{m}") for m in range(4)]
            vsb = [vsbp.tile([128, H, HD + 1], BF16, tag=f"v{tt}", name=f"v{tt}") for tt in range(KC)]
            yt = [ytp.tile([128, T], BF16, tag=f"yt{hp}", name=f"yt{hp}") for hp in range(2)]
            se = [small.tile([97, QT], F32, tag=f"se{qt}", name=f"se{qt}")
                  for qt in range(NQT)]

            # ---- unit emitters -------------------------------------------
            def emit_trans(tt):
                xs = xsbig[tt // 4][:, tt % 4, :]
                for c in range(2):
                    ps = scps.tile([128, 128], BF16, tag="sc", name=f"tp{tt}_{c}")
                    nc.tensor.transpose(ps[:], xs[:, 128 * c:128 * (c + 1)], ident[:])
                    nc.vector.tensor_copy(xt[c][:, 128 * tt:128 * (tt + 1)], ps[:])

            def emit_trans_dma(tt):
                xs = xsbig[tt // 4][:, tt % 4, :]
                for c in range(2):
                    nc.sync.dma_start_transpose(
                        xt[c][:, 128 * tt:128 * (tt + 1)],
                        xs[:, 128 * c:128 * (c + 1)])

            def emit_stage_b(n, m):
                ps = scps.tile([128, QT], F32, tag="sc", name=f"bps{n}_{m}")
                for c in range(2):
                    nc.tensor.matmul(
                        ps[:], w_qk[c][:, 128 * m:128 * (m + 1)],
                        xt[c][:, QT * n:QT * (n + 1)],
                        start=(c == 0), stop=(c == 1))
                nc.vector.tensor_scalar_add(
                    qkt[m][:, QT * n:QT * (n + 1)], ps[:], b_qk[:, m:m + 1])

            def emit_stage_c(i):
                ps = scps.tile([128, C], F32, tag="sc", name=f"cps{i}")
                for c in range(2):
                    nc.tensor.matmul(
                        ps[:], xt[c][:, 128 * i:128 * (i + 1)], w_v[c][:],
                        start=(c == 0), stop=(c == 1))
                nc.vector.tensor_add(
                    vsb[i][:, :, 0:HD],
                    ps[:].rearrange("p (h d) -> p h d", h=H),
                    b_v[:].rearrange("p (h d) -> p h d", h=H))
                nc.vector.tensor_copy(
                    vsb[i][:, :, HD:HD + 1],
                    onescol[:].rearrange("p (h o) -> p h o", o=1))

            rec = [None]

            def emit_rec(qt):
                rec_f = small.tile([97, QT], F32, tag="rec_f")
                nc.vector.reciprocal_approx_fast(rec_f[:], se[qt][:])
                r = small.tile([97, QT], F32R, tag="rec")
                nc.vector.tensor_copy(r[:], rec_f[:])
                rec[0] = r

            def emit_bcmul(qt, hp):
                r = rec[0]
                for h in range(2):
                    p = 32 * (2 * hp + h)
                    bc = scps.tile([HD, QT], F32, tag="sc", name=f"bc{qt}_{hp}{h}")
                    nc.tensor.matmul(bc[:], ones_r[p:p + 1, :], r[p:p + 1, :],
                                     start=True, stop=True,
                                     tile_position=(p, 0))
                    ys = yt[hp][64 * h:64 * (h + 1), QT * qt:QT * (qt + 1)]
                    nc.vector.tensor_mul(ys, ys, bc[:])

            def emit_proj(tt):
                ps = scps.tile([128, C], F32, tag="sc", name=f"pps{tt}")
                for c in range(2):
                    nc.tensor.matmul(
                        ps[:], yt[c][:, 128 * tt:128 * (tt + 1)], w_p[c][:],
                        start=(c == 0), stop=(c == 1))
                ost = ostage.tile([128, C], F32, tag="ost")
                nc.vector.tensor_add(ost[:], ps[:], b_p[:])
                eng = nc.sync if tt % 2 == 0 else nc.gpsimd
                eng.dma_start(out_ap[128 * tt:128 * (tt + 1), :], ost[:])

            # ---- static filler schedule ----------------------------------
            fill = {}

            def add(qt, hp, i, *thunks):
                fill.setdefault((qt, hp, i), []).extend(thunks)

            TR = lambda tt: (lambda: emit_trans(tt))
            SB = lambda n, m: (lambda: emit_stage_b(n, m))
            SC = lambda i: (lambda: emit_stage_c(i))
            RC = lambda qt: (lambda: emit_rec(qt))
            BM = lambda qt, hp: (lambda: emit_bcmul(qt, hp))
            PJ = lambda tt: (lambda: emit_proj(tt))

            add(0, 0, 0, TR(4), SC(1))
            add(0, 0, 1, TR(5), SC(2))
            if not USE_DMA_TRANS:
                add(0, 0, 2, TR(6))
                add(0, 0, 3, TR(7))
                add(0, 0, 4, TR(8), TR(9))
                add(0, 0, 5, TR(10))
                add(0, 0, 6, TR(11))
                add(0, 0, 7, TR(12), TR(13))
                add(0, 0, 8, TR(14), TR(15))
            add(0, 0, 2, SC(3))
            add(0, 0, 3, SB(1, 2), SC(4))
            add(0, 0, 4, SC(5))
            add(0, 0, 5, SC(6))
            add(0, 0, 6, SB(2, 2), SC(7))
            add(0, 0, 7, SC(8))
            add(0, 0, 8, SC(9))
            add(0, 0, 9, SC(10))
            add(0, 0, 10, SB(3, 2), SC(11))
            add(0, 0, 11, SC(12))
            add(0, 0, 12, SC(13), SB(0, 1))
            add(0, 0, 13, SC(14), SB(0, 3))
            add(0, 0, 14, SC(15), SB(1, 3))
            add(0, 1, 2, SB(2, 3))
            add(0, 1, 6, SB(3, 3))
            add(0, 1, 9, SB(1, 0))
            add(0, 1, 12, SB(1, 1))
            for qt in range(1, NQT):
                add(qt, 0, 2, RC(qt - 1))
                add(qt, 0, 3, BM(qt - 1, 0))
                add(qt, 0, 4, BM(qt - 1, 1))
                for j in range(4):
                    add(qt, 0, 5 + 2 * j, PJ(4 * (qt - 1) + j))
                if qt < NQT - 1:
                    add(qt, 0, 13, SB(qt + 1, 0))
                    add(qt, 0, 14, SB(qt + 1, 1))
            # qt=3/hp=0 norm inside the (3,1) stream (se rows 0/32 are
            # written by the (3,0) tail copies, flushed by then)
            add(NQT - 1, 1, 6, RC(NQT - 1))
            add(NQT - 1, 1, 8, BM(NQT - 1, 0))

            # ---- startup prefix ------------------------------------------
            b_qk = consts.tile([128, 4], F32, tag="bqk")
            id4 = consts.tile([4, 4], F32, tag="id4")
            make_identity(nc, id4[:])
            bqps = scps.tile([128, 4], F32, tag="sc", name="bqps")
            nc.tensor.transpose(bqps[:], bqk_nat[:], id4[:])
            nc.vector.tensor_copy(b_qk[:], bqps[:])
            for tt in range(4):
                emit_trans(tt)
            if USE_DMA_TRANS:
                for tt in range(6, KC):
                    emit_trans_dma(tt)
            emit_stage_b(0, 0)
            emit_stage_b(0, 2)
            emit_stage_c(0)

            # ---- main attention stream -----------------------------------
            # pending: thunks to emit after the NEXT group's scores (PV
            # half-matmuls of the previous group + end-of-hp drain copies).
            nsteps = 2 * KC
            pending = []
            o2w = {}                            # h -> writes so far (flags)

            def mk_pv_half(o2t, key, nw, ii, hh_abs, lohi, ext, sl):
                lo = 64 * lohi
                def f():
                    nc.tensor.matmul(
                        o2t[:],
                        vsb[ii][lo:lo + 64, hh_abs, :],
                        ext[lo:lo + 64, QT * sl:QT * (sl + 1)],
                        start=(o2w[key] == 0), stop=(o2w[key] == nw - 1))
                    o2w[key] += 1
                return f

            def mk_pv_full(o2t, key, nw, ii, hh_abs, ext, sl):
                def f():
                    nc.tensor.matmul(
                        o2t[:],
                        vsb[ii][:, hh_abs, :],
                        ext[:, QT * sl:QT * (sl + 1)],
                        start=(o2w[key] == 0), stop=(o2w[key] == nw - 1))
                    o2w[key] += 1
                return f

            for qt in range(NQT):
                for hp in range(2):
                    qT = qkt[hp]
                    kT = qkt[hp + 2]
                    o2 = None
                    # per-group step lists -> count writes per o2 bank:
                    # full chunks inside a group contribute 2 half-MMs per
                    # step; steps split across groups contribute 1 full MM.
                    groups = []
                    s = 0
                    while s < nsteps:
                        g = list(range(s, min(s + GROUP, nsteps)))
                        groups.append(g)
                        s += GROUP
                    nwrites = {0: 0, 1: 0}
                    for g in groups:
                        k = 0
                        while k < len(g):
                            s0 = g[k]
                            if (SPLIT_PV and k + 1 < len(g) and s0 % 2 == 0):
                                nwrites[0] += 2
                                nwrites[1] += 2
                                k += 2
                            else:
                                nwrites[divmod(s0, 2)[1]] += 1
                                k += 1
                    key0, key1 = (qt, hp, 0), (qt, hp, 1)
                    o2w[key0] = o2w[key1] = 0

                    for gi, g in enumerate(groups):
                        for s in g:
                            i, h = divmod(s, 2)
                            if h == 0:
                                for f in fill.get((qt, hp, i), []):
                                    f()
                            if s == g[0]:
                                sc_t = scps.tile([128, GROUP * QT], F32,
                                                 tag="sc", name=f"sc{qt}{hp}_{s}")
                            nc.tensor.matmul(
                                sc_t[:, QT * (s - g[0]):QT * (s - g[0] + 1)],
                                kT[64 * h:64 * (h + 1), 128 * i:128 * (i + 1)],
                                qT[64 * h:64 * (h + 1), QT * qt:QT * (qt + 1)],
                                start=True, stop=True)
                        # flush previous group's PV work (and, at an hp
                        # boundary, the previous hp's drain copies) BEFORE
                        # this group's exp: keeps the PE fed while ACT runs
                        # and orders the drains ahead of the new o2 alloc.
                        for f in pending:
                            f()
                        pending = []
                        if o2 is None:
                            o2 = [o2ps.tile([HD + 1, QT], F32, tag=f"o2{h}",
                                            name=f"o2_{qt}{hp}{h}")
                                  for h in range(2)]
                        ns = len(g)
                        ex = expp.tile([128, GROUP * QT], BF16, tag="ex")
                        nc.scalar.activation(
                            ex[:, :QT * ns], sc_t[:, :QT * ns],
                            mybir.ActivationFunctionType.Exp,
                            bias=0.0, scale=float(HD) ** -0.5)
                        # PV work for this group: (A||D),(B||C) pattern for
                        # paired steps; lone steps get a full-K matmul.
                        k = 0
                        while k < len(g):
                            s0 = g[k]
                            i0, h0 = divmod(s0, 2)
                            sl0 = s0 - g[0]
                            if SPLIT_PV and k + 1 < len(g) and s0 % 2 == 0:
                                s1 = g[k + 1]
                                sl1 = s1 - g[0]
                                ha, hb = 2 * hp, 2 * hp + 1
                                pending.append(mk_pv_half(o2[0], key0, nwrites[0], i0, ha, 0, ex, sl0))
                                pending.append(mk_pv_half(o2[1], key1, nwrites[1], i0, hb, 1, ex, sl1))
                                pending.append(mk_pv_half(o2[0], key0, nwrites[0], i0, ha, 1, ex, sl0))
                                pending.append(mk_pv_half(o2[1], key1, nwrites[1], i0, hb, 0, ex, sl1))
                                k += 2
                            else:
                                pending.append(mk_pv_full(
                                    o2[h0], (qt, hp, h0), nwrites[h0], i0,
                                    2 * hp + h0, ex, sl0))
                                k += 1

                    def mk_drain(qt_, hp_, o2_):
                        def f():
                            for h in range(2):
                                nc.vector.tensor_copy(
                                    yt[hp_][64 * h:64 * (h + 1),
                                            QT * qt_:QT * (qt_ + 1)],
                                    o2_[h][0:HD, :])
                                nc.vector.tensor_copy(
                                    se[qt_][32 * (2 * hp_ + h):32 * (2 * hp_ + h) + 1, :],
                                    o2_[h][HD:HD + 1, :])
                        return f
                    pending.append(mk_drain(qt, hp, o2))

            for f in pending:
                f()

            # ---- tail: hp=1 norm + proj of the last q-tile, pipelined in
            # 128-col subchunks so the first store issues ~2us earlier.
            emit_rec(NQT - 1)
            r_tail = rec[0]
            for sub in range(4):
                tt = 4 * (NQT - 1) + sub
                for h in range(2):
                    p = 32 * (2 + h)
                    bc = scps.tile([HD, 128], F32, tag="sc", name=f"tbc{sub}{h}")
                    nc.tensor.matmul(
                        bc[:], ones_r[p:p + 1, :],
                        r_tail[p:p + 1, 128 * sub:128 * (sub + 1)],
                        start=True, stop=True, tile_position=(p, 0))
                    ys = yt[1][64 * h:64 * (h + 1), 128 * tt:128 * (tt + 1)]
                    nc.vector.tensor_mul(ys, ys, bc[:])
                emit_proj(tt)
    nc.compile()
    return nc


_NC_CACHE = []


def _get_nc():
    if not _NC_CACHE:
        _NC_CACHE.append(build_nc())
    return _NC_CACHE[0]


def make_in_maps(x, w_qkv, b_qkv, w_proj, b_proj):
    shared = {
        "w_qkT": np.ascontiguousarray(w_qkv[:2 * C].T, dtype=np.float32),
        "w_vT": np.ascontiguousarray(w_qkv[2 * C:].T, dtype=np.float32),
        "w_pT": np.ascontiguousarray(w_proj.T, dtype=np.float32),
        "b_qk": np.ascontiguousarray(b_qkv[:2 * C].reshape(4, 128), dtype=np.float32),
        "b_v": np.ascontiguousarray(b_qkv[2 * C:], dtype=np.float32),
        "b_p": np.ascontiguousarray(b_proj, dtype=np.float32),
    }
    return [dict(shared, x=np.ascontiguousarray(x[b], dtype=np.float32))
            for b in range(B)]


def run(x, w_qkv, b_qkv, w_proj, b_proj, trace=False):
    nc = _get_nc()
    in_maps = make_in_maps(np.asarray(x), np.asarray(w_qkv), np.asarray(b_qkv),
                           np.asarray(w_proj), np.asarray(b_proj))
    res = run_bass_kernel_spmd(nc, in_maps, list(range(N_CORES)), trace=trace)
    out = np.stack([res.results[b]["out"] for b in range(B)])
    return out, res


def kernel(x, w_qkv, b_qkv, w_proj, b_proj):
    out, _ = run(x, w_qkv, b_qkv, w_proj, b_proj, trace=False)
    return out
